# revision 33
# baseline (speedup 1.0000x reference)
"""Trainium2 Bass kernel for nn_MeshCrossAttention (mesh cross-attention + per-head MLP).

Sharding: data-parallel over batch B=16 -> 2 batches per NeuronCore, 8 cores,
no collectives.

v2 design (vs v1 baseline at ~1.33 ms):
  - bf16 operands everywhere on the PE (fp32 PSUM accumulate). Halves DMA and
    SBUF traffic; all projection weights stay RESIDENT in SBUF (loaded once).
  - Transposed projections exactly like v1 (qT/kT via lhsT=W^T chunks), V in
    natural head-interleaved layout va [LK, 4, H*(HD+1)] with a ones column.
  - Scores stay transposed (sT [LK, LQ]; lhsT = kT head slice), exp on ScalarE
    -> eT bf16 tiles.
  - Context is accumulated in NATURAL layout: ctx[LQ, j*(HD+1)] via
    lhsT = eT chunk [LK, LQ-chunk], rhs = va slice [LK, HD+1]. The ones column
    of va makes column HD the softmax denominator, which now lives PER
    PARTITION -> normalization is a plain DVE reciprocal + tensor_scalar
    multiply. No DRAM-roundtrip partition broadcast (v1's big serializer).
  - cat [LQ, 192] is transposed back with PE identity-matmuls for the per-head
    MLP (contraction over 192 needs cat^T), then MLP1 -> Gelu -> MLP2 with the
    MLP2 output written naturally into a [LQ, D] staging tile -> single DMA out.
  - Exp and Gelu are batched in 8-head phases so the ScalarE activation table
    swaps 4x per batch instead of 52x.
  - PSUM budget: big(2) + att(2) + ctx(3, paired [128, 390]) + m64(1) = 8 banks.
"""
import math
import sys

import numpy as np

if "/opt/trn_rl_repo" not in sys.path:
    sys.path.insert(0, "/opt/trn_rl_repo")

import ml_dtypes  # noqa: E402

import concourse.bass as bass  # noqa: E402
import concourse.tile as tile  # noqa: E402
from concourse import bacc, mybir  # noqa: E402
from concourse.bass_utils import run_bass_kernel_spmd  # noqa: E402

F32 = mybir.dt.float32
BF16 = mybir.dt.bfloat16

D, H, HD, J = 1024, 16, 64, 3
B, LQ, LK = 16, 512, 512
P = 128
N_CORES = 8
B_LOC = B // N_CORES  # 2
E = HD + 1            # 65: head stripe width in va (ones column at HD)
HG = 8                # heads per exp/gelu phase group


def _emit(tc, aps, dbg=False):
    nc = tc.nc
    ctx_mgr = []

    def pool(name, bufs, space="SBUF"):
        p = tc.tile_pool(name=name, bufs=bufs, space=space)
        ctx_mgr.append(p)
        return p.__enter__()

    const = pool("const", 1)
    ain = pool("ain", 12)          # streamed activation chunks [128, 512] bf16
    qt_pool = pool("qt", 24)       # zero-padded per-head qT tiles [128, 512]
    kt_pool = pool("kt", 24)
    va_pool = pool("va", 3)
    expp = pool("expp", 10)
    recp = pool("recp", 4)
    catp = pool("catp", 10)        # pair cat tiles [128, 384]
    ctp = pool("ctp", 14)          # catTj pair tiles, alive until mini-MLP
    h1p = pool("h1p", 4)
    ostg = pool("ostg", 5)

    ps_big = pool("ps_big", 2, "PSUM")   # proj accum + cat transposes + mlp1
    ps_att = pool("ps_att", 3, "PSUM")   # scores / mlp2 [128, 512]
    ps_ctx = pool("ps_ctx", 3, "PSUM")   # ctx pairs [128, 2*J*E = 390]

    # ---------------- resident constants ----------------
    # Only wq/bq are DMAed up front (the first projection needs them); the
    # rest is deferred until after the Q-projection is emitted so the PE can
    # start ~8us earlier at kernel start.
    wq_sb, wk_sb, wv_sb = [], [], []
    for nm, lst in (("wqt", wq_sb), ("wkt", wk_sb), ("wvt", wv_sb)):
        for i in range(8):
            t = const.tile([P, D], BF16, tag=f"{nm}{i}", name=f"{nm}{i}")
            lst.append(t)
    for i in range(8):
        nc.sync.dma_start(out=wq_sb[i][:], in_=aps["wqt"][i * P:(i + 1) * P, :])
    bq_sb = const.tile([P, 8], F32, tag="bq", name="bq_sb")
    nc.sync.dma_start(out=bq_sb[:], in_=aps["bq"][:, :])

    w1jd = [const.tile([P, P], BF16, tag=f"w1jd{j}", name=f"w1jd{j}")
            for j in range(J)]
    w2bd = const.tile([P, P], BF16, tag="w2bd", name="w2bd")
    ident = const.tile([P, P], BF16, tag="ident", name="ident")
    bk_sb = const.tile([P, 8], F32, tag="bk", name="bk_sb")
    bv_bc = const.tile([P, D], BF16, tag="bv", name="bv_bc")
    b2_bc = const.tile([P, D], F32, tag="b2", name="b2_bc")
    b1_sb = const.tile([P, 1], F32, tag="b1", name="b1_sb")
    tbl_scr = const.tile([P, 1], F32, tag="tbl", name="tbl_scr")

    def deferred_k_dmas():
        for i in range(8):
            nc.sync.dma_start(out=wk_sb[i][:],
                              in_=aps["wkt"][i * P:(i + 1) * P, :])
        nc.sync.dma_start(out=bk_sb[:], in_=aps["bk"][:, :])

    def deferred_v_dmas():
        for i in range(8):
            nc.sync.dma_start(out=wv_sb[i][:],
                              in_=aps["wvt"][i * P:(i + 1) * P, :])
        nc.sync.dma_start(out=bv_bc[:], in_=aps["bv_bc"][:, :])

    def deferred_mlp_dmas():
        for j in range(J):
            nc.sync.dma_start(out=w1jd[j][:], in_=aps["w1jd"][j])
        nc.sync.dma_start(out=w2bd[:], in_=aps["w2bd"][:, :])
        nc.sync.dma_start(out=ident[:], in_=aps["ident"][:, :])
        nc.sync.dma_start(out=b2_bc[:], in_=aps["b2_bc"][:, :])
        nc.sync.dma_start(out=b1_sb[:], in_=aps["b1"][:, :])

    def load_acts(ap_slice):
        ts = []
        for ic in range(8):
            t = ain.tile([P, 512], BF16, tag="ain", name="act")
            nc.sync.dma_start(out=t[:], in_=ap_slice[ic * P:(ic + 1) * P, :])
            ts.append(t)
        return ts

    def proj_T(w_tiles, x_tiles, bias_sb, out_pool, out_tag):
        """out[oc] [128, 512] = (W @ x^T) chunk + bias, bf16."""
        outs = []
        for oc in range(8):
            pss = ps_big.tile([P, 512], F32, tag="big", name="pss")
            for ic in range(8):
                nc.tensor.matmul(
                    out=pss[:], lhsT=w_tiles[ic][:, oc * P:(oc + 1) * P],
                    rhs=x_tiles[ic][:], start=(ic == 0), stop=(ic == 7))
            t = out_pool.tile([P, 512], BF16, tag=out_tag, name=out_tag)
            nc.vector.tensor_scalar_add(t[:], pss[:], bias_sb[:, oc:oc + 1])
            outs.append(t)
        return outs

    def proj_Q(x_tiles):
        """Q projection into zero-padded per-head tiles qtp[h] [128, 512]:
        head h's 64 q-dims at rows (h%2)*64, the other 64 rows ZERO, so the
        scores matmul is a full 128x128x512 tile against the compact kT
        (the zero q rows null the paired head's k contribution)."""
        outs = []
        for oc in range(8):
            pss = ps_big.tile([P, 512], F32, tag="big", name="pss")
            for ic in range(8):
                nc.tensor.matmul(
                    out=pss[:], lhsT=wq_sb[ic][:, oc * P:(oc + 1) * P],
                    rhs=x_tiles[ic][:], start=(ic == 0), stop=(ic == 7))
            te = qt_pool.tile([P, 512], BF16, tag="qt", name="qtp_e")
            to = qt_pool.tile([P, 512], BF16, tag="qt", name="qtp_o")
            nc.gpsimd.memset(te[HD:P, :], 0.0)
            nc.gpsimd.memset(to[0:HD, :], 0.0)
            nc.vector.tensor_scalar_add(te[0:HD, :], pss[0:HD, :],
                                        bq_sb[0:HD, oc:oc + 1])
            nc.vector.tensor_scalar_add(to[HD:P, :], pss[HD:P, :],
                                        bq_sb[HD:P, oc:oc + 1])
            outs += [te, to]
        return outs

    def proj_V(x_tiles, va):
        """va [128, 4, H*E] natural head-interleaved V + ones column."""
        nc.sync.dma_start(
            out=va.rearrange("p c (h e) -> p c h e", e=E)[:, :, :, HD],
            in_=aps["ones_cols"][:, :, :])
        for half in range(2):
            for nck in range(4):
                pss = ps_big.tile([P, 512], F32, tag="big", name="pssv")
                for ic in range(8):
                    nc.tensor.matmul(
                        out=pss[:],
                        lhsT=x_tiles[ic][:, nck * P:(nck + 1) * P],
                        rhs=wv_sb[ic][:, half * 512:(half + 1) * 512],
                        start=(ic == 0), stop=(ic == 7))
                dst = va[:, nck, :].rearrange("p (h e) -> p h e", e=E)[
                    :, half * 8:(half + 1) * 8, 0:HD]
                nc.vector.tensor_tensor(
                    out=dst,
                    in0=pss[:].rearrange("p (h e) -> p h e", e=HD),
                    in1=bv_bc[:, half * 512:(half + 1) * 512].rearrange(
                        "p (h e) -> p h e", e=HD),
                    op=mybir.AluOpType.add)

    for b in range(B_LOC):
        # ================= projections =================
        qin = load_acts(aps["qt_in"][b])
        if b == 0:
            deferred_k_dmas()
        qtp = proj_Q(qin)

        kT = []
        for j in range(J):
            kin = load_acts(aps["kt_in"][j, b])
            if b == 0 and j == 0:
                deferred_v_dmas()
            kT.append(proj_T(wk_sb, kin, bk_sb, kt_pool, "kt"))

        va_list = []
        for j in range(J):
            vin = load_acts(aps["vt_in"][j, b])
            if b == 0 and j == 0:
                deferred_mlp_dmas()
            va = va_pool.tile([P, 4, H * E], BF16, tag="va", name="va")
            proj_V(vin, va)
            va_list.append(va)
        # pull the Exp table in while the PE finishes projections
        nc.scalar.activation(out=tbl_scr[:], in_=b1_sb[:],
                             func=mybir.ActivationFunctionType.Exp)

        if dbg and b == 0:
            for oc in range(8):
                nc.sync.dma_start(out=aps["dbg_qt"][oc], in_=qtp[oc][:])
                nc.sync.dma_start(out=aps["dbg_kt0"][oc], in_=kT[0][oc][:])
            nc.sync.dma_start(out=aps["dbg_va0"][:, :, :], in_=va_list[0][:])

        ost = [ostg.tile([P, D], F32, tag="ostg", name=f"ost{i}")
               for i in range(4)]

        # ================= attention + MLP, 8-head phases =================
        # Per head: 12 (score -> exp) steps; ctx matmuls consume the PREVIOUS
        # step's exp output so the PE never waits on ScalarE. The previous
        # head's cat transposes are interleaved as PE filler during exp
        # latency. Every PE matmul in this phase is a full 128x128 tile
        # (partial-K/M matmuls measure ~2x slower on HW): scores use the
        # zero-padded kT, the j2 cat columns are packed per head PAIR and
        # MLP1/MLP2 use block-diagonal weights over head pairs.
        for hg in range(H // HG):
            catT_all = {}
            catp_tiles = None
            fillers = []
            for h in range(hg * HG, (hg + 1) * HG):
                pscs = [ps_ctx.tile([P, 2 * J * E], F32, tag="ctx",
                                    name=f"psc{i}") for i in range(2)]

                def emit_ctx(j, ci, et):
                    for lqc in range(4):
                        # One start/stop per PSUM bank: start lazily zeroes
                        # the whole 2KB bank, so only the first matmul
                        # touching each pair-tile starts the group.
                        nc.tensor.matmul(
                            out=pscs[lqc // 2][
                                :, (lqc % 2) * J * E + j * E:
                                (lqc % 2) * J * E + (j + 1) * E],
                            lhsT=et[:, lqc * P:(lqc + 1) * P],
                            rhs=va_list[j][:, ci, h * E:(h + 1) * E],
                            start=(j == 0 and ci == 0 and lqc % 2 == 0),
                            stop=(j == J - 1 and ci == 3 and lqc % 2 == 1))

                pend = []
                for j in range(J):
                    for ci in range(4):
                        pss = ps_att.tile([P, LQ], F32, tag="att", name="ps_s")
                        nc.tensor.matmul(
                            out=pss[:],
                            lhsT=kT[j][h // 2][:, ci * P:(ci + 1) * P],
                            rhs=qtp[h][:], start=True, stop=True)
                        et = expp.tile([P, LQ], BF16, tag="expp", name="et")
                        nc.scalar.activation(
                            out=et[:], in_=pss[:],
                            func=mybir.ActivationFunctionType.Exp,
                            scale=1.0 / math.sqrt(HD))
                        if dbg and b == 0 and h == 0 and j == 0:
                            nc.sync.dma_start(out=aps["dbg_exp"][ci], in_=et[:])
                        if fillers:
                            fillers.pop(0)()
                        pend.append((j, ci, et))
                        if len(pend) > 2:
                            emit_ctx(*pend.pop(0))
                while pend:
                    emit_ctx(*pend.pop(0))

                # normalize: denominator is column HD of each head stripe.
                # Each j-chunk goes into the head PAIR's shared catp tile
                # (64 columns per head) so the transposes and MLP1 run as
                # full 128x128 tiles with block-diagonal weights.
                if h % 2 == 0:
                    catp_tiles = [catp.tile([P, J * P], BF16, tag="catp",
                                            name="catp") for _ in range(4)]
                for pi in range(2):
                    rec = recp.tile([P, 2 * J], F32, tag="rec", name="rec")
                    nc.vector.reciprocal(
                        rec[:],
                        pscs[pi].rearrange("p (x e) -> p x e", e=E)[:, :, HD])
                    for half in range(2):
                        lqc = pi * 2 + half
                        # one fused multiply: [128, 3, 64] x per-(row,j)
                        # reciprocal broadcast along the last dim
                        nc.vector.tensor_tensor(
                            out=catp_tiles[lqc].rearrange(
                                "p (x e) -> p x e", e=P)[
                                :, :, (h % 2) * HD:(h % 2) * HD + HD],
                            in0=pscs[pi].rearrange(
                                "p (x e) -> p x e", e=E)[
                                :, half * J:(half + 1) * J, 0:HD],
                            in1=rec[:, half * J:(half + 1) * J].unsqueeze(2)
                            .to_broadcast((P, J, HD)),
                            op=mybir.AluOpType.mult)
                    if dbg and b == 0 and h == 0:
                        nc.sync.dma_start(out=aps["dbg_rec"][pi], in_=rec[:])
                if dbg and b == 0 and h == 1:
                    for lqc in range(4):
                        nc.sync.dma_start(out=aps["dbg_cat"][lqc],
                                          in_=catp_tiles[lqc][:, 0:P])

                # After the odd head, queue the pair's transpose work as PE
                # filler for the next head's exp-latency slots.
                if h % 2 == 1:
                    thunks = []
                    catTs = []
                    for j in range(J):
                        ptj = ps_big.tile([P, LQ], F32, tag="big", name="ptj")
                        catTj = ctp.tile([P, LQ], BF16, tag="ct", name="catTj")
                        for lqc in range(4):
                            thunks.append(
                                lambda lqc=lqc, ptj=ptj, j=j,
                                c=catp_tiles[lqc]:
                                nc.tensor.matmul(
                                    out=ptj[:, lqc * P:(lqc + 1) * P],
                                    lhsT=c[:, j * P:(j + 1) * P], rhs=ident[:],
                                    start=(lqc == 0), stop=(lqc == 3)))
                        thunks.append(
                            lambda ptj=ptj, catTj=catTj:
                            nc.vector.tensor_copy(out=catTj[:], in_=ptj[:]))
                        catTs.append(catTj)
                    catT_all[h // 2] = catTs
                    fillers.extend(thunks)
            while fillers:
                fillers.pop(0)()

            # ---- MLP1 / Gelu / MLP2 per head pair, full 128-tiles ----
            # tiny dummy activation pulls the Gelu table in while the PE is
            # still busy with transposes, instead of stalling the first gelu
            nc.scalar.activation(out=tbl_scr[:], in_=b1_sb[:],
                                 func=mybir.ActivationFunctionType.Gelu)
            for pi in range(hg * HG // 2, (hg + 1) * HG // 2):
                ph1p = ps_big.tile([P, LQ], F32, tag="big", name="ph1p")
                for j in range(J):
                    nc.tensor.matmul(out=ph1p[:, :], lhsT=w1jd[j][:],
                                     rhs=catT_all[pi][j][:],
                                     start=(j == 0), stop=(j == J - 1))
                h1 = h1p.tile([P, LQ], BF16, tag="h1", name="h1")
                nc.scalar.activation(
                    out=h1[:], in_=ph1p[:],
                    func=mybir.ActivationFunctionType.Gelu, bias=b1_sb[:])
                if dbg and b == 0 and pi == 0:
                    nc.sync.dma_start(out=aps["dbg_catT0"][:, :],
                                      in_=catT_all[0][0][:])
                    nc.sync.dma_start(out=aps["dbg_h1"][:, :], in_=h1[:])
                for lqc in range(4):
                    ps2 = ps_att.tile([P, LQ], F32, tag="att", name="ps2")
                    nc.tensor.matmul(
                        out=ps2[:, 0:P], lhsT=h1[:, lqc * P:(lqc + 1) * P],
                        rhs=w2bd[:], start=True, stop=True)
                    nc.vector.tensor_add(
                        ost[lqc][:, pi * P:(pi + 1) * P], ps2[:, 0:P],
                        b2_bc[:, pi * P:(pi + 1) * P])
            # preload the Exp table for the next attention phase
            nc.scalar.activation(out=tbl_scr[:], in_=b1_sb[:],
                                 func=mybir.ActivationFunctionType.Exp)
            # stream out this head-group's half of the output columns
            for lqc in range(4):
                nc.sync.dma_start(
                    out=aps["out"][b, lqc * P:(lqc + 1) * P,
                                   hg * 512:(hg + 1) * 512],
                    in_=ost[lqc][:, hg * 512:(hg + 1) * 512])


    for p in reversed(ctx_mgr):
        p.__exit__(None, None, None)


_CACHE = {}


def _build(dbg=False):
    key = ("nc", dbg)
    if key in _CACHE:
        return _CACHE[key]
    nc = bacc.Bacc("TRN2", target_bir_lowering=False, debug=False)
    shapes = {
        "qt_in": ([B_LOC, D, LQ], BF16),
        "kt_in": ([J, B_LOC, D, LK], BF16),
        "vt_in": ([J, B_LOC, D, LK], BF16),
        "wqt": ([D, D], BF16),
        "wkt": ([D, D], BF16),
        "wvt": ([D, D], BF16),
        "w1jd": ([J, P, P], BF16),
        "w2bd": ([P, P], BF16),
        "ident": ([P, P], BF16),
        "ones_cols": ([P, 4, H], BF16),
        "bq": ([P, 8], F32),
        "bk": ([P, 8], F32),
        "bv_bc": ([P, D], BF16),
        "b2_bc": ([P, D], F32),
        "b1": ([P, 1], F32),
    }
    aps = {k: nc.dram_tensor(k, s, dt, kind="ExternalInput").ap()
           for k, (s, dt) in shapes.items()}
    aps["out"] = nc.dram_tensor("out", [B_LOC, LQ, D], F32,
                                kind="ExternalOutput").ap()
    if dbg:
        dbg_shapes = {
            "dbg_qt": ([8, P, 512], BF16), "dbg_kt0": ([8, P, 512], BF16),
            "dbg_va0": ([P, 4, H * E], BF16), "dbg_exp": ([4, P, LQ], BF16),
            "dbg_rec": ([2, P, 2 * J], F32), "dbg_cat": ([4, P, P], BF16),
            "dbg_catT0": ([P, LQ], BF16), "dbg_h1": ([P, LQ], BF16),
        }
        for k, (shp, dt) in dbg_shapes.items():
            aps[k] = nc.dram_tensor(k, shp, dt, kind="ExternalOutput").ap()
    with tile.TileContext(nc) as tc:
        _emit(tc, aps, dbg=dbg)
    nc.compile()
    _CACHE[key] = nc
    return nc


def _prep_in_maps(inputs):
    f32 = np.float32
    bf16 = ml_dtypes.bfloat16
    q = np.ascontiguousarray(np.asarray(inputs["query_states"], f32))
    k = np.ascontiguousarray(np.asarray(inputs["key_states"], f32))
    v = np.ascontiguousarray(np.asarray(inputs["value_states"], f32))
    Wq = np.asarray(inputs["Wq"], f32)
    Wk = np.asarray(inputs["Wk"], f32)
    Wv = np.asarray(inputs["Wv"], f32)
    W1 = np.asarray(inputs["W1"], f32)
    W2 = np.asarray(inputs["W2"], f32)
    bq = np.asarray(inputs["bq"], f32)
    bk = np.asarray(inputs["bk"], f32)
    bv = np.asarray(inputs["bv"], f32)
    b1 = np.asarray(inputs["b1"], f32)
    b2 = np.asarray(inputs["b2"], f32)

    wqt = np.ascontiguousarray(Wq.T).astype(bf16)
    wkt = np.ascontiguousarray(Wk.T).astype(bf16)
    wvt = np.ascontiguousarray(Wv.T).astype(bf16)
    W1T = np.ascontiguousarray(W1.T)                       # [192, 64]
    w1jd = np.zeros((J, P, P), f32)
    for j in range(J):
        blk = W1T[j * HD:(j + 1) * HD]                     # [64, 64]
        w1jd[j, :HD, :HD] = blk
        w1jd[j, HD:, HD:] = blk
    w1jd = w1jd.astype(bf16)
    W2T = np.ascontiguousarray(W2.T)                       # [64, 64]
    w2bd = np.zeros((P, P), f32)
    w2bd[:HD, :HD] = W2T
    w2bd[HD:, HD:] = W2T
    w2bd = w2bd.astype(bf16)
    ident = np.eye(P, dtype=f32).astype(bf16)
    bq_sb = np.ascontiguousarray(bq.reshape(8, P).T).astype(f32)
    bk_sb = np.ascontiguousarray(bk.reshape(8, P).T).astype(f32)
    bv_bc = np.tile(bv, (P, 1)).astype(bf16)
    b2_bc = np.tile(b2, (P, H)).astype(f32)
    b1_col = np.concatenate([b1, b1]).reshape(P, 1).astype(f32)
    ones_cols = np.ones((P, 4, H), f32).astype(bf16)

    qt_all = np.ascontiguousarray(q.transpose(0, 2, 1)).astype(bf16)
    kt_all = np.ascontiguousarray(k.transpose(0, 1, 3, 2)).astype(bf16)
    vt_all = np.ascontiguousarray(v.transpose(0, 1, 3, 2)).astype(bf16)

    in_maps = []
    for c in range(N_CORES):
        sl = slice(c * B_LOC, (c + 1) * B_LOC)
        in_maps.append({
            "qt_in": np.ascontiguousarray(qt_all[sl]),
            "kt_in": np.ascontiguousarray(kt_all[:, sl]),
            "vt_in": np.ascontiguousarray(vt_all[:, sl]),
            "wqt": wqt, "wkt": wkt, "wvt": wvt,
            "w1jd": w1jd, "w2bd": w2bd, "ident": ident,
            "ones_cols": ones_cols,
            "bq": bq_sb, "bk": bk_sb, "bv_bc": bv_bc,
            "b2_bc": b2_bc, "b1": b1_col,
        })
    return in_maps


def kernel(**inputs):
    nc = _build()
    in_maps = _prep_in_maps(inputs)
    res = run_bass_kernel_spmd(nc, in_maps, core_ids=list(range(N_CORES)))
    out = np.concatenate([res.results[i]["out"] for i in range(N_CORES)], axis=0)
    return out.astype(np.float32)


# revision 34
# speedup vs baseline: 1.0347x; 1.0347x over previous
"""Trainium2 Bass kernel for nn_MeshCrossAttention (mesh cross-attention + per-head MLP).

Sharding: data-parallel over batch B=16 -> 2 batches per NeuronCore, 8 cores,
no collectives.

v2 design (vs v1 baseline at ~1.33 ms):
  - bf16 operands everywhere on the PE (fp32 PSUM accumulate). Halves DMA and
    SBUF traffic; all projection weights stay RESIDENT in SBUF (loaded once).
  - Transposed projections exactly like v1 (qT/kT via lhsT=W^T chunks), V in
    natural head-interleaved layout va [LK, 4, H*(HD+1)] with a ones column.
  - Scores stay transposed (sT [LK, LQ]; lhsT = kT head slice), exp on ScalarE
    -> eT bf16 tiles.
  - Context is accumulated in NATURAL layout: ctx[LQ, j*(HD+1)] via
    lhsT = eT chunk [LK, LQ-chunk], rhs = va slice [LK, HD+1]. The ones column
    of va makes column HD the softmax denominator, which now lives PER
    PARTITION -> normalization is a plain DVE reciprocal + tensor_scalar
    multiply. No DRAM-roundtrip partition broadcast (v1's big serializer).
  - cat [LQ, 192] is transposed back with PE identity-matmuls for the per-head
    MLP (contraction over 192 needs cat^T), then MLP1 -> Gelu -> MLP2 with the
    MLP2 output written naturally into a [LQ, D] staging tile -> single DMA out.
  - Exp and Gelu are batched in 8-head phases so the ScalarE activation table
    swaps 4x per batch instead of 52x.
  - PSUM budget: big(2) + att(2) + ctx(3, paired [128, 390]) + m64(1) = 8 banks.
"""
import math
import sys

import numpy as np

if "/opt/trn_rl_repo" not in sys.path:
    sys.path.insert(0, "/opt/trn_rl_repo")

import ml_dtypes  # noqa: E402

import concourse.bass as bass  # noqa: E402
import concourse.tile as tile  # noqa: E402
from concourse import bacc, mybir  # noqa: E402
from concourse.bass_utils import run_bass_kernel_spmd  # noqa: E402

F32 = mybir.dt.float32
BF16 = mybir.dt.bfloat16

D, H, HD, J = 1024, 16, 64, 3
B, LQ, LK = 16, 512, 512
P = 128
N_CORES = 8
B_LOC = B // N_CORES  # 2
E = HD + 1            # 65: head stripe width in va (ones column at HD)
HG = 8                # heads per exp/gelu phase group


def _emit(tc, aps, dbg=False):
    nc = tc.nc
    ctx_mgr = []

    def pool(name, bufs, space="SBUF"):
        p = tc.tile_pool(name=name, bufs=bufs, space=space)
        ctx_mgr.append(p)
        return p.__enter__()

    const = pool("const", 1)
    ain = pool("ain", 12)          # streamed activation chunks [128, 512] bf16
    qt_pool = pool("qt", 24)       # zero-padded per-head qT tiles [128, 512]
    kt_pool = pool("kt", 24)
    va_pool = pool("va", 3)
    expp = pool("expp", 10)
    recp = pool("recp", 4)
    catp = pool("catp", 10)        # pair cat tiles [128, 384]
    ctp = pool("ctp", 14)          # catTj pair tiles, alive until mini-MLP
    h1p = pool("h1p", 4)
    ostg = pool("ostg", 5)

    ps_big = pool("ps_big", 2, "PSUM")   # proj accum + cat transposes + mlp1
    ps_att = pool("ps_att", 3, "PSUM")   # scores / mlp2 [128, 512]
    ps_ctx = pool("ps_ctx", 3, "PSUM")   # ctx pairs [128, 2*J*E = 390]

    # ---------------- resident constants ----------------
    # Only wq/bq are DMAed up front (the first projection needs them); the
    # rest is deferred until after the Q-projection is emitted so the PE can
    # start ~8us earlier at kernel start.
    wq_sb, wk_sb, wv_sb = [], [], []
    for nm, lst in (("wqt", wq_sb), ("wkt", wk_sb), ("wvt", wv_sb)):
        for i in range(8):
            t = const.tile([P, D], BF16, tag=f"{nm}{i}", name=f"{nm}{i}")
            lst.append(t)
    for i in range(8):
        nc.sync.dma_start(out=wq_sb[i][:], in_=aps["wqt"][i * P:(i + 1) * P, :])
    bq_sb = const.tile([P, 8], F32, tag="bq", name="bq_sb")
    nc.sync.dma_start(out=bq_sb[:], in_=aps["bq"][:, :])

    w1jd = [const.tile([P, P], BF16, tag=f"w1jd{j}", name=f"w1jd{j}")
            for j in range(J)]
    w2bd = const.tile([P, P], BF16, tag="w2bd", name="w2bd")
    ident = const.tile([P, P], BF16, tag="ident", name="ident")
    bk_sb = const.tile([P, 8], F32, tag="bk", name="bk_sb")
    bv_bc = const.tile([P, D], BF16, tag="bv", name="bv_bc")
    b2_bc = const.tile([P, D], F32, tag="b2", name="b2_bc")
    b1_sb = const.tile([P, 1], F32, tag="b1", name="b1_sb")
    tbl_scr = const.tile([P, 1], F32, tag="tbl", name="tbl_scr")

    def deferred_const_dmas():
        for i in range(8):
            nc.sync.dma_start(out=wk_sb[i][:],
                              in_=aps["wkt"][i * P:(i + 1) * P, :])
        nc.sync.dma_start(out=bk_sb[:], in_=aps["bk"][:, :])
        for i in range(8):
            nc.sync.dma_start(out=wv_sb[i][:],
                              in_=aps["wvt"][i * P:(i + 1) * P, :])
        nc.sync.dma_start(out=bv_bc[:], in_=aps["bv_bc"][:, :])
        for j in range(J):
            nc.sync.dma_start(out=w1jd[j][:], in_=aps["w1jd"][j])
        nc.sync.dma_start(out=w2bd[:], in_=aps["w2bd"][:, :])
        nc.sync.dma_start(out=ident[:], in_=aps["ident"][:, :])
        nc.sync.dma_start(out=b2_bc[:], in_=aps["b2_bc"][:, :])
        nc.sync.dma_start(out=b1_sb[:], in_=aps["b1"][:, :])

    def load_acts(ap_slice):
        ts = []
        for ic in range(8):
            t = ain.tile([P, 512], BF16, tag="ain", name="act")
            nc.sync.dma_start(out=t[:], in_=ap_slice[ic * P:(ic + 1) * P, :])
            ts.append(t)
        return ts

    def proj_T(w_tiles, x_tiles, bias_sb, out_pool, out_tag):
        """out[oc] [128, 512] = (W @ x^T) chunk + bias, bf16."""
        outs = []
        for oc in range(8):
            pss = ps_big.tile([P, 512], F32, tag="big", name="pss")
            for ic in range(8):
                nc.tensor.matmul(
                    out=pss[:], lhsT=w_tiles[ic][:, oc * P:(oc + 1) * P],
                    rhs=x_tiles[ic][:], start=(ic == 0), stop=(ic == 7))
            t = out_pool.tile([P, 512], BF16, tag=out_tag, name=out_tag)
            nc.vector.tensor_scalar_add(t[:], pss[:], bias_sb[:, oc:oc + 1])
            outs.append(t)
        return outs

    def proj_Q(x_tiles):
        """Q projection into zero-padded per-head tiles qtp[h] [128, 512]:
        head h's 64 q-dims at rows (h%2)*64, the other 64 rows ZERO, so the
        scores matmul is a full 128x128x512 tile against the compact kT
        (the zero q rows null the paired head's k contribution)."""
        outs = []
        for oc in range(8):
            pss = ps_big.tile([P, 512], F32, tag="big", name="pss")
            for ic in range(8):
                nc.tensor.matmul(
                    out=pss[:], lhsT=wq_sb[ic][:, oc * P:(oc + 1) * P],
                    rhs=x_tiles[ic][:], start=(ic == 0), stop=(ic == 7))
            te = qt_pool.tile([P, 512], BF16, tag="qt", name="qtp_e")
            to = qt_pool.tile([P, 512], BF16, tag="qt", name="qtp_o")
            nc.gpsimd.memset(te[HD:P, :], 0.0)
            nc.gpsimd.memset(to[0:HD, :], 0.0)
            nc.vector.tensor_scalar_add(te[0:HD, :], pss[0:HD, :],
                                        bq_sb[0:HD, oc:oc + 1])
            nc.vector.tensor_scalar_add(to[HD:P, :], pss[HD:P, :],
                                        bq_sb[HD:P, oc:oc + 1])
            outs += [te, to]
        return outs

    def proj_V(x_tiles, va):
        """va [128, 4, H*E] natural head-interleaved V + ones column."""
        nc.sync.dma_start(
            out=va.rearrange("p c (h e) -> p c h e", e=E)[:, :, :, HD],
            in_=aps["ones_cols"][:, :, :])
        for half in range(2):
            for nck in range(4):
                pss = ps_big.tile([P, 512], F32, tag="big", name="pssv")
                for ic in range(8):
                    nc.tensor.matmul(
                        out=pss[:],
                        lhsT=x_tiles[ic][:, nck * P:(nck + 1) * P],
                        rhs=wv_sb[ic][:, half * 512:(half + 1) * 512],
                        start=(ic == 0), stop=(ic == 7))
                dst = va[:, nck, :].rearrange("p (h e) -> p h e", e=E)[
                    :, half * 8:(half + 1) * 8, 0:HD]
                nc.vector.tensor_tensor(
                    out=dst,
                    in0=pss[:].rearrange("p (h e) -> p h e", e=HD),
                    in1=bv_bc[:, half * 512:(half + 1) * 512].rearrange(
                        "p (h e) -> p h e", e=HD),
                    op=mybir.AluOpType.add)

    for b in range(B_LOC):
        # ================= projections =================
        qin = load_acts(aps["qt_in"][b])
        qtp = proj_Q(qin)
        if b == 0:
            deferred_const_dmas()

        kT = []
        for j in range(J):
            kin = load_acts(aps["kt_in"][j, b])
            kT.append(proj_T(wk_sb, kin, bk_sb, kt_pool, "kt"))

        va_list = []
        for j in range(J):
            vin = load_acts(aps["vt_in"][j, b])
            va = va_pool.tile([P, 4, H * E], BF16, tag="va", name="va")
            proj_V(vin, va)
            va_list.append(va)

        if dbg and b == 0:
            for oc in range(8):
                nc.sync.dma_start(out=aps["dbg_qt"][oc], in_=qtp[oc][:])
                nc.sync.dma_start(out=aps["dbg_kt0"][oc], in_=kT[0][oc][:])
            nc.sync.dma_start(out=aps["dbg_va0"][:, :, :], in_=va_list[0][:])

        ost = [ostg.tile([P, D], F32, tag="ostg", name=f"ost{i}")
               for i in range(4)]

        # ================= attention + MLP, 8-head phases =================
        # Per head: 12 (score -> exp) steps; ctx matmuls consume the PREVIOUS
        # step's exp output so the PE never waits on ScalarE. The previous
        # head's cat transposes are interleaved as PE filler during exp
        # latency. Every PE matmul in this phase is a full 128x128 tile
        # (partial-K/M matmuls measure ~2x slower on HW): scores use the
        # zero-padded kT, the j2 cat columns are packed per head PAIR and
        # MLP1/MLP2 use block-diagonal weights over head pairs.
        for hg in range(H // HG):
            catT_all = {}
            catp_tiles = None
            fillers = []
            for h in range(hg * HG, (hg + 1) * HG):
                pscs = [ps_ctx.tile([P, 2 * J * E], F32, tag="ctx",
                                    name=f"psc{i}") for i in range(2)]

                def emit_ctx(j, ci, et):
                    for lqc in range(4):
                        # One start/stop per PSUM bank: start lazily zeroes
                        # the whole 2KB bank, so only the first matmul
                        # touching each pair-tile starts the group.
                        nc.tensor.matmul(
                            out=pscs[lqc // 2][
                                :, (lqc % 2) * J * E + j * E:
                                (lqc % 2) * J * E + (j + 1) * E],
                            lhsT=et[:, lqc * P:(lqc + 1) * P],
                            rhs=va_list[j][:, ci, h * E:(h + 1) * E],
                            start=(j == 0 and ci == 0 and lqc % 2 == 0),
                            stop=(j == J - 1 and ci == 3 and lqc % 2 == 1))

                pend = []
                for j in range(J):
                    for ci in range(4):
                        pss = ps_att.tile([P, LQ], F32, tag="att", name="ps_s")
                        nc.tensor.matmul(
                            out=pss[:],
                            lhsT=kT[j][h // 2][:, ci * P:(ci + 1) * P],
                            rhs=qtp[h][:], start=True, stop=True)
                        et = expp.tile([P, LQ], BF16, tag="expp", name="et")
                        nc.scalar.activation(
                            out=et[:], in_=pss[:],
                            func=mybir.ActivationFunctionType.Exp,
                            scale=1.0 / math.sqrt(HD))
                        if dbg and b == 0 and h == 0 and j == 0:
                            nc.sync.dma_start(out=aps["dbg_exp"][ci], in_=et[:])
                        if fillers:
                            fillers.pop(0)()
                        pend.append((j, ci, et))
                        if len(pend) > 2:
                            emit_ctx(*pend.pop(0))
                while pend:
                    emit_ctx(*pend.pop(0))

                # normalize: denominator is column HD of each head stripe.
                # Each j-chunk goes into the head PAIR's shared catp tile
                # (64 columns per head) so the transposes and MLP1 run as
                # full 128x128 tiles with block-diagonal weights.
                if h % 2 == 0:
                    catp_tiles = [catp.tile([P, J * P], BF16, tag="catp",
                                            name="catp") for _ in range(4)]
                for pi in range(2):
                    rec = recp.tile([P, 2 * J], F32, tag="rec", name="rec")
                    nc.vector.reciprocal(
                        rec[:],
                        pscs[pi].rearrange("p (x e) -> p x e", e=E)[:, :, HD])
                    for half in range(2):
                        lqc = pi * 2 + half
                        # one fused multiply: [128, 3, 64] x per-(row,j)
                        # reciprocal broadcast along the last dim
                        nc.vector.tensor_tensor(
                            out=catp_tiles[lqc].rearrange(
                                "p (x e) -> p x e", e=P)[
                                :, :, (h % 2) * HD:(h % 2) * HD + HD],
                            in0=pscs[pi].rearrange(
                                "p (x e) -> p x e", e=E)[
                                :, half * J:(half + 1) * J, 0:HD],
                            in1=rec[:, half * J:(half + 1) * J].unsqueeze(2)
                            .to_broadcast((P, J, HD)),
                            op=mybir.AluOpType.mult)
                    if dbg and b == 0 and h == 0:
                        nc.sync.dma_start(out=aps["dbg_rec"][pi], in_=rec[:])
                if dbg and b == 0 and h == 1:
                    for lqc in range(4):
                        nc.sync.dma_start(out=aps["dbg_cat"][lqc],
                                          in_=catp_tiles[lqc][:, 0:P])

                # After the odd head, queue the pair's transpose work as PE
                # filler for the next head's exp-latency slots.
                if h % 2 == 1:
                    thunks = []
                    catTs = []
                    for j in range(J):
                        ptj = ps_big.tile([P, LQ], F32, tag="big", name="ptj")
                        catTj = ctp.tile([P, LQ], BF16, tag="ct", name="catTj")
                        for lqc in range(4):
                            thunks.append(
                                lambda lqc=lqc, ptj=ptj, j=j,
                                c=catp_tiles[lqc]:
                                nc.tensor.matmul(
                                    out=ptj[:, lqc * P:(lqc + 1) * P],
                                    lhsT=c[:, j * P:(j + 1) * P], rhs=ident[:],
                                    start=(lqc == 0), stop=(lqc == 3)))
                        thunks.append(
                            lambda ptj=ptj, catTj=catTj:
                            nc.vector.tensor_copy(out=catTj[:], in_=ptj[:]))
                        catTs.append(catTj)
                    catT_all[h // 2] = catTs
                    fillers.extend(thunks)
            while fillers:
                fillers.pop(0)()

            # ---- MLP1 / Gelu / MLP2 per head pair, full 128-tiles ----
            # tiny dummy activation pulls the Gelu table in while the PE is
            # still busy with transposes, instead of stalling the first gelu
            nc.scalar.activation(out=tbl_scr[:], in_=b1_sb[:],
                                 func=mybir.ActivationFunctionType.Gelu)
            for pi in range(hg * HG // 2, (hg + 1) * HG // 2):
                ph1p = ps_big.tile([P, LQ], F32, tag="big", name="ph1p")
                for j in range(J):
                    nc.tensor.matmul(out=ph1p[:, :], lhsT=w1jd[j][:],
                                     rhs=catT_all[pi][j][:],
                                     start=(j == 0), stop=(j == J - 1))
                h1 = h1p.tile([P, LQ], BF16, tag="h1", name="h1")
                nc.scalar.activation(
                    out=h1[:], in_=ph1p[:],
                    func=mybir.ActivationFunctionType.Gelu, bias=b1_sb[:])
                if dbg and b == 0 and pi == 0:
                    nc.sync.dma_start(out=aps["dbg_catT0"][:, :],
                                      in_=catT_all[0][0][:])
                    nc.sync.dma_start(out=aps["dbg_h1"][:, :], in_=h1[:])
                for lqc in range(4):
                    ps2 = ps_att.tile([P, LQ], F32, tag="att", name="ps2")
                    nc.tensor.matmul(
                        out=ps2[:, 0:P], lhsT=h1[:, lqc * P:(lqc + 1) * P],
                        rhs=w2bd[:], start=True, stop=True)
                    nc.vector.tensor_add(
                        ost[lqc][:, pi * P:(pi + 1) * P], ps2[:, 0:P],
                        b2_bc[:, pi * P:(pi + 1) * P])
            # preload the Exp table for the next attention phase
            nc.scalar.activation(out=tbl_scr[:], in_=b1_sb[:],
                                 func=mybir.ActivationFunctionType.Exp)
            # stream out this head-group's half of the output columns
            for lqc in range(4):
                nc.sync.dma_start(
                    out=aps["out"][b, lqc * P:(lqc + 1) * P,
                                   hg * 512:(hg + 1) * 512],
                    in_=ost[lqc][:, hg * 512:(hg + 1) * 512])


    for p in reversed(ctx_mgr):
        p.__exit__(None, None, None)


_CACHE = {}


def _build(dbg=False):
    key = ("nc", dbg)
    if key in _CACHE:
        return _CACHE[key]
    nc = bacc.Bacc("TRN2", target_bir_lowering=False, debug=False)
    shapes = {
        "qt_in": ([B_LOC, D, LQ], BF16),
        "kt_in": ([J, B_LOC, D, LK], BF16),
        "vt_in": ([J, B_LOC, D, LK], BF16),
        "wqt": ([D, D], BF16),
        "wkt": ([D, D], BF16),
        "wvt": ([D, D], BF16),
        "w1jd": ([J, P, P], BF16),
        "w2bd": ([P, P], BF16),
        "ident": ([P, P], BF16),
        "ones_cols": ([P, 4, H], BF16),
        "bq": ([P, 8], F32),
        "bk": ([P, 8], F32),
        "bv_bc": ([P, D], BF16),
        "b2_bc": ([P, D], F32),
        "b1": ([P, 1], F32),
    }
    aps = {k: nc.dram_tensor(k, s, dt, kind="ExternalInput").ap()
           for k, (s, dt) in shapes.items()}
    aps["out"] = nc.dram_tensor("out", [B_LOC, LQ, D], F32,
                                kind="ExternalOutput").ap()
    if dbg:
        dbg_shapes = {
            "dbg_qt": ([8, P, 512], BF16), "dbg_kt0": ([8, P, 512], BF16),
            "dbg_va0": ([P, 4, H * E], BF16), "dbg_exp": ([4, P, LQ], BF16),
            "dbg_rec": ([2, P, 2 * J], F32), "dbg_cat": ([4, P, P], BF16),
            "dbg_catT0": ([P, LQ], BF16), "dbg_h1": ([P, LQ], BF16),
        }
        for k, (shp, dt) in dbg_shapes.items():
            aps[k] = nc.dram_tensor(k, shp, dt, kind="ExternalOutput").ap()
    with tile.TileContext(nc) as tc:
        _emit(tc, aps, dbg=dbg)
    nc.compile()
    _CACHE[key] = nc
    return nc


def _prep_in_maps(inputs):
    f32 = np.float32
    bf16 = ml_dtypes.bfloat16
    q = np.ascontiguousarray(np.asarray(inputs["query_states"], f32))
    k = np.ascontiguousarray(np.asarray(inputs["key_states"], f32))
    v = np.ascontiguousarray(np.asarray(inputs["value_states"], f32))
    Wq = np.asarray(inputs["Wq"], f32)
    Wk = np.asarray(inputs["Wk"], f32)
    Wv = np.asarray(inputs["Wv"], f32)
    W1 = np.asarray(inputs["W1"], f32)
    W2 = np.asarray(inputs["W2"], f32)
    bq = np.asarray(inputs["bq"], f32)
    bk = np.asarray(inputs["bk"], f32)
    bv = np.asarray(inputs["bv"], f32)
    b1 = np.asarray(inputs["b1"], f32)
    b2 = np.asarray(inputs["b2"], f32)

    wqt = np.ascontiguousarray(Wq.T).astype(bf16)
    wkt = np.ascontiguousarray(Wk.T).astype(bf16)
    wvt = np.ascontiguousarray(Wv.T).astype(bf16)
    W1T = np.ascontiguousarray(W1.T)                       # [192, 64]
    w1jd = np.zeros((J, P, P), f32)
    for j in range(J):
        blk = W1T[j * HD:(j + 1) * HD]                     # [64, 64]
        w1jd[j, :HD, :HD] = blk
        w1jd[j, HD:, HD:] = blk
    w1jd = w1jd.astype(bf16)
    W2T = np.ascontiguousarray(W2.T)                       # [64, 64]
    w2bd = np.zeros((P, P), f32)
    w2bd[:HD, :HD] = W2T
    w2bd[HD:, HD:] = W2T
    w2bd = w2bd.astype(bf16)
    ident = np.eye(P, dtype=f32).astype(bf16)
    bq_sb = np.ascontiguousarray(bq.reshape(8, P).T).astype(f32)
    bk_sb = np.ascontiguousarray(bk.reshape(8, P).T).astype(f32)
    bv_bc = np.tile(bv, (P, 1)).astype(bf16)
    b2_bc = np.tile(b2, (P, H)).astype(f32)
    b1_col = np.concatenate([b1, b1]).reshape(P, 1).astype(f32)
    ones_cols = np.ones((P, 4, H), f32).astype(bf16)

    qt_all = np.ascontiguousarray(q.transpose(0, 2, 1)).astype(bf16)
    kt_all = np.ascontiguousarray(k.transpose(0, 1, 3, 2)).astype(bf16)
    vt_all = np.ascontiguousarray(v.transpose(0, 1, 3, 2)).astype(bf16)

    in_maps = []
    for c in range(N_CORES):
        sl = slice(c * B_LOC, (c + 1) * B_LOC)
        in_maps.append({
            "qt_in": np.ascontiguousarray(qt_all[sl]),
            "kt_in": np.ascontiguousarray(kt_all[:, sl]),
            "vt_in": np.ascontiguousarray(vt_all[:, sl]),
            "wqt": wqt, "wkt": wkt, "wvt": wvt,
            "w1jd": w1jd, "w2bd": w2bd, "ident": ident,
            "ones_cols": ones_cols,
            "bq": bq_sb, "bk": bk_sb, "bv_bc": bv_bc,
            "b2_bc": b2_bc, "b1": b1_col,
        })
    return in_maps


def kernel(**inputs):
    nc = _build()
    in_maps = _prep_in_maps(inputs)
    res = run_bass_kernel_spmd(nc, in_maps, core_ids=list(range(N_CORES)))
    out = np.concatenate([res.results[i]["out"] for i in range(N_CORES)], axis=0)
    return out.astype(np.float32)
